# revision 76
# baseline (speedup 1.0000x reference)
"""Trainium2 Bass kernel for a ReActNet-style binary BasicBlock.

Full inputs: x [32,256,28,28] f32 + params. Data-parallel over batch across
8 NeuronCores (4 images per core, params replicated, no collectives).

Key algebra (forward pass only):
  _binact(x)  == sign(x)            (STE wrapper vanishes in forward)
  _binweight(w) == mean|w|_o * sign(w)
So each conv is a +-1 x +-1 matmul: exact in fp8e4m3 with fp32 PSUM
accumulation (integer partial sums <= 2304 << 2^24). fp8 enables DoubleRow
matmuls (K=256 per instruction, 2x PE throughput). Per-channel weight scale
and BN affine fold into one multiply-add applied to the PSUM result.

Layout: channels on partitions, 2 channel-blocks of 128 interleaved as the
DoubleRow pair dimension. 3x3 conv = 9 shifted matmuls over a zero-padded
30x30 frame; each matmul streams a contiguous 418-column window (14 padded
rows) and the 392 valid pixels are extracted by the strided PSUM read of the
following fused (s*psum + residual) op.
"""

import numpy as np
import ml_dtypes

import concourse.bacc as bacc
import concourse.mybir as mybir
from concourse.tile import TileContext
from concourse.bass_utils import run_bass_kernel_spmd

F32 = mybir.dt.float32
FP8 = mybir.dt.float8e4
AF = mybir.ActivationFunctionType
OP = mybir.AluOpType
DR = mybir.MatmulPerfMode.DoubleRow

NCORES = 8
P = 128
C = 256
HB = 4            # images per core
H = W = 28
HP = WP = 30      # padded frame
NPIX = HB * H * W    # 3136
IMG = H * W          # 784
CH = IMG // 2        # 392-pixel chunk (half image) per PSUM bank
SEAM = 13 * WP + W   # 418: contiguous window covering 14 padded rows

# consts layout: [128, 2*10] -> per-channel scalars, one column each per block.
# J_SG2B: threshold for sign2 computed directly from u_pre — sign(prelu(t)+c2)
# is monotone in t, so it equals [u_pre + (t1b - t*) >= 0] with t* = prelu^-1(-c2)
# J_NB11 (= -b11): threshold form of sign1 for the DVE fast path at startup
J_B11, J_S1, J_T1B, J_A1, J_SG2B, J_S2, J_T2B, J_A2, J_B23, J_NB11 = range(10)
NCN = 10

_CACHE = {}

X_SPLIT = False      # image-0 load in two row bands
DVE_SIGN1 = False    # image-0 block-1 sign on DVE + per-block x0 DMA
LAST_ADD_DVE = False  # final-group +b23 on DVE instead of GpSimd
GROUPS_CFG = [(0,), (1, 2), (3,)]  # images per weight-reuse group

# emission schedule: ("c1", grp, ob) | ("c2", grp) | ("e1"/"e2", grp, fine)
SCHEDULE = [
    ("c1", 0, 0), ("c1", 0, 1), ("e1", 0, False),
    ("c1", 1, 0, True), ("c2", 0), ("e2", 0, False),
    ("c1", 1, 1, True), ("e1", 1, False),
    ("c1", 2, 0), ("c2", 1), ("e2", 1, False),
    ("c1", 2, 1, True), ("e1", 2, True),
    ("c2", 2), ("e2", 2, True),
]


def _build_nc():
    nc = bacc.Bacc(None, target_bir_lowering=False, debug=False)

    x_t = nc.dram_tensor("x", [HB, C, H, W], F32, kind="ExternalInput")
    w1_t = nc.dram_tensor("w1t", [P, 9 * 2 * C], FP8, kind="ExternalInput")
    w2_t = nc.dram_tensor("w2t", [P, 2 * C], FP8, kind="ExternalInput")
    c_t = nc.dram_tensor("consts", [P, 2 * NCN], F32, kind="ExternalInput")
    out_t = nc.dram_tensor("out", [HB, C, H, W], F32, kind="ExternalOutput")

    with TileContext(nc) as tc:
        with tc.tile_pool(name="main", bufs=1) as pool, \
             tc.tile_pool(name="ps", bufs=1, space="PSUM") as psp:

            def ptile(nm, shape, dt):
                return pool.tile(shape, dt, name=nm, tag=nm)

            c_sb = ptile("c_sb", [P, 2 * NCN], F32)
            x_sb = ptile("x_sb", [P, 2 * NPIX], F32)  # channel-block major
            pad_sb = ptile("pad_sb", [P, 2 * HB * HP * WP], FP8)
            w1_sb = ptile("w1_sb", [P, 9 * 2 * C], FP8)
            w2_sb = ptile("w2_sb", [P, 2 * C], FP8)
            sg2_sb = ptile("sg2_sb", [P, 2 * NPIX], FP8)
            u_pre = [ptile(f"u_pre{i}", [P, NPIX], F32) for i in range(2)]
            p1 = [ptile(f"p1_{i}", [P, NPIX], F32) for i in range(2)]
            v_pre = [ptile(f"v_pre{i}", [P, NPIX], F32) for i in range(2)]
            p2 = [ptile(f"p2_{i}", [P, NPIX], F32) for i in range(2)]

            def cc(blk, j):  # per-channel scalar AP [128,1]
                return c_sb[:, blk * NCN + j:blk * NCN + j + 1]

            def xsl(blk, sl):  # x slice [128, n] for channel block blk
                return x_sb[:, blk * NPIX + sl.start:blk * NPIX + sl.stop]

            # ---- loads, ordered so image 0 can start ASAP (consts ride the
            # scalar engine's HWDGE stream, off the critical sync queue) ----
            nc.scalar.dma_start(out=c_sb, in_=c_t[:])
            xv4 = x_sb.rearrange("p (j b q) -> p j b q", j=2, b=HB)
            xrows0 = x_sb.rearrange("p (j b h w) -> p j b h w",
                                    j=2, b=HB, h=H, w=W)
            # image 0 in two row-bands so Sign (and the first matmuls) can
            # start before the whole image lands
            x0_in = x_t[0].rearrange("(j p) h w -> p j h w", p=P)
            if X_SPLIT:
                nc.sync.dma_start(out=xrows0[:, :, 0, 0:15],
                                  in_=x0_in[:, :, 0:15])
                nc.sync.dma_start(out=xrows0[:, :, 0, 15:H],
                                  in_=x0_in[:, :, 15:H])
            elif DVE_SIGN1:
                # per channel block, so each block's Sign starts as soon as
                # its half lands (block 1 runs on DVE, in parallel with ACT)
                for j in range(2):
                    nc.sync.dma_start(
                        out=xv4[:, j, 0],
                        in_=x_t[0, j * P:(j + 1) * P].rearrange(
                            "c h w -> c (h w)"))
            else:
                nc.sync.dma_start(out=xv4[:, :, 0],
                                  in_=x_t[0].rearrange(
                                      "(j p) h w -> p j (h w)", p=P))
            # first three kernel positions land first: the opening matmuls
            # only need w1v[:, 0..2] while the rest of the weights stream in
            nc.sync.dma_start(out=w1_sb[:, 0:3 * 2 * C], in_=w1_t[:, 0:3 * 2 * C])
            nc.sync.dma_start(out=w1_sb[:, 3 * 2 * C:], in_=w1_t[:, 3 * 2 * C:])
            for j in range(2):
                nc.sync.dma_start(
                    out=xv4[:, j, 1:HB],
                    in_=x_t[1:HB, j * P:(j + 1) * P].rearrange(
                        "b c h w -> c b (h w)"))
            nc.sync.dma_start(out=w2_sb, in_=w2_t[:])

            # warm the ACT function-table set (Copy shares the set with
            # Sign/Prelu) so the ~2.7us table load overlaps the input DMAs
            dummy = ptile("dummy", [P, 8], F32)
            nc.vector.memset(dummy[:, 0:4], 0.0)
            nc.scalar.activation(dummy[:, 4:8], dummy[:, 0:4], AF.Copy)

            pad5 = pad_sb.rearrange("p (j b h w) -> p j b h w",
                                    j=2, b=HB, h=HP, w=WP)
            pad_j = pad_sb.rearrange("p (j q) -> p j q", j=2)
            w1v = w1_sb.rearrange("p (k j o) -> p k j o", k=9, j=2)
            w2v = w2_sb.rearrange("p (j o) -> p j o", j=2)
            sg2v = sg2_sb.rearrange("p (j q) -> p j q", j=2)

            # zero only the pad borders, per image (interiors are overwritten
            # by Sign): top+bottom rows, then left+right columns
            for b in range(HB):
                for i in range(2):
                    nc.vector.memset(pad5[:, i, b, 0:HP:HP - 1, :], 0.0)
                    nc.vector.memset(pad5[:, i, b, 1:HP - 1, 0:WP:WP - 1], 0.0)

            # sgn1 = Sign(x + b11) written into padded interior, fp8.
            # Image 0 in two row-bands matching its split DMA.
            if X_SPLIT:
                for i in range(2):
                    nc.scalar.activation(
                        pad5[:, i, 0, 1:16, 1:1 + W], xrows0[:, i, 0, 0:15],
                        AF.Sign, bias=cc(i, J_B11))
                for i in range(2):
                    nc.scalar.activation(
                        pad5[:, i, 0, 16:1 + H, 1:1 + W], xrows0[:, i, 0, 15:H],
                        AF.Sign, bias=cc(i, J_B11))
            if not X_SPLIT and DVE_SIGN1:
                # image 0, block 0 on ACT; block 1 on DVE as
                # 2*[x >= -b11] - 1 so both blocks sign concurrently
                nc.scalar.activation(
                    pad5[:, 0, 0, 1:1 + H, 1:1 + W],
                    xsl(0, slice(0, IMG)).rearrange("p (h w) -> p h w", h=H),
                    AF.Sign, bias=cc(0, J_B11))
                sgntmp = ptile("sgntmp", [P, IMG], F32)
                nc.vector.tensor_scalar(sgntmp, xsl(1, slice(0, IMG)),
                                        cc(1, J_NB11), None, op0=OP.is_ge)
                nc.vector.tensor_scalar(
                    pad5[:, 1, 0, 1:1 + H, 1:1 + W],
                    sgntmp.rearrange("p (h w) -> p h w", h=H),
                    2.0, -1.0, op0=OP.mult, op1=OP.add)
            b0_range = range(1, HB) if (X_SPLIT or DVE_SIGN1) else range(HB)
            for b in b0_range:
                for i in range(2):
                    nc.scalar.activation(
                        pad5[:, i, b, 1:1 + H, 1:1 + W],
                        xsl(i, slice(b * IMG, (b + 1) * IMG)).rearrange(
                            "p (h w) -> p h w", h=H),
                        AF.Sign, bias=cc(i, J_B11))

            # chunk groups sharing stationary weights: singleton first/last
            # groups shorten the kernel head (image 0 prep only) and tail
            GROUPS = GROUPS_CFG

            def grp_chunks(grp):
                return [(b, half) for b in GROUPS[grp] for half in range(2)]

            def chunk_sl(b, half):
                return slice(b * IMG + half * CH, b * IMG + half * CH + CH)

            def r14x28(ap_flat_392):
                return ap_flat_392.rearrange("p (r c) -> p r c", c=W)

            def conv1_group(grp, ob, seq=False):
                chunks = grp_chunks(grp)
                # ps_0/ps_1 double-buffer across groups; ps_2/ps_3 are only
                # used by the 4-chunk middle groups (single-buffered) so two
                # banks stay free for conv2's dedicated tiles
                pss = [psp.tile([P, 420], F32, name=f"ps_{ci}",
                                tag=f"ps_{ci}", bufs=(2 if ci < 2 else 1))
                       for ci in range(len(chunks))]
                # seq=True: finish chunks one at a time (extra LDWEIGHTS, but
                # the per-chunk epilogue chain starts a weight-pass earlier)
                chunk_sets = ([[c] for c in chunks] if seq else [chunks])
                for cset in chunk_sets:
                    for k in range(9):
                        ky, kx = divmod(k, 3)
                        w = w1v[:, k, :, ob * P:(ob + 1) * P]
                        for b, half in cset:
                            ci = chunks.index((b, half))
                            off = b * HP * WP + (half * (H // 2) + ky) * WP + kx
                            nc.tensor.matmul(
                                pss[ci][:, 0:SEAM], w,
                                pad_j[:, :, off:off + SEAM],
                                start=(k == 0), stop=(k == 8), perf_mode=DR)
                for ci, (b, half) in enumerate(chunks):
                    # u_pre = s1 * psum + x  (valid 392 of the 418 columns)
                    psv = pss[ci].rearrange("p (r c) -> p r c", c=WP)[:, 0:14, 0:W]
                    sl = chunk_sl(b, half)
                    nc.vector.scalar_tensor_tensor(
                        r14x28(u_pre[ob][:, sl]), psv, cc(ob, J_S1),
                        r14x28(xsl(ob, sl)), op0=OP.mult, op1=OP.add)

            def branch1_ew(grp, fine):
                # branch-2 activation as a {0,1} indicator on GpSimd, straight
                # from u_pre (prelu1 is off this path): conv2(2u-1) is
                # recovered via s2'=2*s2 and the host-folded rowsum in t2b'.
                # Always chunk-grained: it gates the next conv2 group.
                for b, half in grp_chunks(grp):
                    sl = chunk_sl(b, half)
                    for ob in range(2):
                        nc.gpsimd.tensor_scalar(
                            sg2v[:, ob, sl], u_pre[ob][:, sl],
                            cc(ob, J_SG2B), 0.0, op0=OP.add, op1=OP.is_ge)
                # p1 = prelu(u_pre + (bn1_const + b12), a1): residual only
                if fine:
                    pieces = [chunk_sl(b, half) for b, half in grp_chunks(grp)]
                else:
                    pieces = [slice(b * IMG, (b + 1) * IMG) for b in GROUPS[grp]]
                for sl in pieces:
                    for ob in range(2):
                        nc.scalar.activation(p1[ob][:, sl], u_pre[ob][:, sl],
                                             AF.Prelu, bias=cc(ob, J_T1B),
                                             alpha=cc(ob, J_A1))

            def conv2_group(grp):
                chunks = grp_chunks(grp)
                for ob in range(2):
                    w = w2v[:, :, ob * P:(ob + 1) * P]
                    pss = [psp.tile([P, 420], F32, name=f"cps_{ci % 2}",
                                    tag=f"cps_{ci % 2}", bufs=1)
                           for ci in range(len(chunks))]
                    for ci, (b, half) in enumerate(chunks):
                        nc.tensor.matmul(
                            pss[ci][:, 0:CH], w, sg2v[:, :, chunk_sl(b, half)],
                            start=True, stop=True, perf_mode=DR)
                    for ci, (b, half) in enumerate(chunks):
                        # v_pre = s2 * psum + p1
                        sl = chunk_sl(b, half)
                        nc.vector.scalar_tensor_tensor(
                            v_pre[ob][:, sl], pss[ci][:, 0:CH], cc(ob, J_S2),
                            p1[ob][:, sl], op0=OP.mult, op1=OP.add)

            def branch2_ew(grp, fine):
                pieces = (grp_chunks(grp) if fine
                          else [(b, None) for b in GROUPS[grp]])
                for b, half in pieces:
                    sl = chunk_sl(b, half) if half is not None else \
                        slice(b * IMG, (b + 1) * IMG)
                    for ob in range(2):
                        # p2 = prelu(v_pre + (bn2_const + b22 + b13), a2)
                        nc.scalar.activation(
                            p2[ob][:, sl], v_pre[ob][:, sl],
                            AF.Prelu, bias=cc(ob, J_T2B),
                            alpha=cc(ob, J_A2))
                        # out = p2 + b23, staged into x_sb (x is dead by now)
                        eng = (nc.vector if LAST_ADD_DVE
                               and grp == len(GROUPS) - 1 else nc.gpsimd)
                        eng.tensor_scalar_add(xsl(ob, sl),
                                              p2[ob][:, sl], cc(ob, J_B23))
                    hw0 = 0 if half is None else half * (H // 2)
                    nrows = H if half is None else H // 2
                    if half is None:
                        # one DMA per image covering both channel blocks
                        xq = x_sb.rearrange("p (j q) -> p j q", j=2)
                        nc.sync.dma_start(
                            out=out_t[b, :, hw0:hw0 + nrows].rearrange(
                                "(j p) h w -> p j (h w)", p=P),
                            in_=xq[:, :, sl.start:sl.stop])
                    else:
                        # tail pieces ship per channel block, as soon as ready
                        for ob in range(2):
                            nc.sync.dma_start(
                                out=out_t[b, ob * P:(ob + 1) * P,
                                          hw0:hw0 + nrows].rearrange(
                                    "c h w -> c (h w)"),
                                in_=xsl(ob, sl))

            # schedule: each conv2 group is sandwiched inside the next conv1
            # group so the PE never waits on the elementwise chain; the final
            # group (image 3 only) runs fine-grained (392-px pieces) to
            # shorten the kernel tail.
            ops = {
                "c1": conv1_group,
                "c2": conv2_group,
                "e1": branch1_ew,
                "e2": branch2_ew,
            }
            for step in SCHEDULE:
                ops[step[0]](*step[1:])

    _dedup_ldweights(nc)
    nc.compile()
    return nc


def _dedup_ldweights(nc):
    """Drop InstLdweights that reload the exact weights already resident in
    the PE array (weight-reuse groups emit one matmul per pixel chunk, and the
    tile lowering pairs every matmul with its own Ldweights). Only wait-free,
    update-free duplicates are removed, so semaphore semantics are untouched."""
    pe = nc.tensor.engine
    removed = 0
    for blk in nc.m.functions[0].blocks:
        last_sig = None
        keep = []
        for ins in blk.instructions:
            if ins.engine == pe:
                t = type(ins).__name__
                if t == "InstLdweights":
                    sig = repr(ins.ins[0])
                    if (sig == last_sig and not ins.has_wait()
                            and not ins.has_update()):
                        removed += 1
                        continue
                    last_sig = sig
                elif t not in ("InstMatmult", "InstEventSemaphore"):
                    last_sig = None
            keep.append(ins)
        if removed:
            while len(blk.instructions):
                blk.instructions.pop()
            for ins in keep:
                blk.instructions.append(ins)
    return removed


def _prep_params(w1, w2, b11, b12, b13, b21, b22, b23,
                 g1, be1, m1, v1, g2, be2, m2, v2, a1, a2):
    f = np.float32
    eps = f(1e-5)
    scale1 = np.mean(np.abs(w1), axis=(1, 2, 3), dtype=np.float32)
    scale2 = np.mean(np.abs(w2), axis=(1, 2, 3), dtype=np.float32)
    inv1 = (g1 / np.sqrt(v1 + eps)).astype(f)
    inv2 = (g2 / np.sqrt(v2 + eps)).astype(f)
    s1 = (scale1 * inv1).astype(f)
    s2 = (scale2 * inv2).astype(f)
    t1b = ((be1 - m1 * inv1) + b12).astype(f)
    c2 = (b13 + b21).astype(f)
    t2b = (((be2 - m2 * inv2) + b22) + b13).astype(f)
    # branch-2 activations are sent as indicators u in {0,1}:
    # conv2(2u-1)[o] = 2*conv2(u)[o] - rowsum(sign(w2))[o]
    rs2 = np.sign(w2[:, :, 0, 0]).astype(f).sum(axis=1, dtype=np.float32)
    s2_eff = (2.0 * s2).astype(f)
    t2b_eff = (t2b - s2 * rs2).astype(f)
    # sign2 threshold straight from u_pre: sign(prelu(t)+c2) = [t >= t*],
    # t* = -c2 (c2<=0) else -c2/a1  (prelu is monotone for a1>0)
    a1_safe = np.where(a1 != 0, a1, f(1.0)).astype(f)
    t_star = np.where(c2 <= 0, -c2, -(c2 / a1_safe)).astype(f)
    sg2b = (t1b - t_star).astype(f)

    consts = np.stack([b11, s1, t1b, a1, sg2b, s2_eff, t2b_eff, a2, b23,
                       -b11], axis=1).astype(f).reshape(2, P, NCN)
    consts = np.ascontiguousarray(consts.transpose(1, 0, 2).reshape(P, 2 * NCN))

    fp8 = ml_dtypes.float8_e4m3
    # w1t[p, k, j, o] = sign(w1)[o, j*128+p, ky, kx]
    w1t = np.sign(w1).astype(np.float32).transpose(1, 2, 3, 0)  # [I,ky,kx,O]
    w1t = w1t.reshape(2, P, 9, C).transpose(1, 2, 0, 3)          # [p,k,j,o]
    w1t = np.ascontiguousarray(w1t.reshape(P, 9 * 2 * C)).astype(fp8)
    # w2t[p, j, o] = sign(w2)[o, j*128+p]
    w2t = np.sign(w2[:, :, 0, 0]).astype(np.float32).T.reshape(2, P, C)
    w2t = np.ascontiguousarray(w2t.transpose(1, 0, 2).reshape(P, 2 * C)).astype(fp8)
    return consts, w1t, w2t


def kernel(x, loss, b11, b12, b13, b21, b22, b23, w1, w2,
           g1, be1, m1, v1, g2, be2, m2, v2, a1, a2):
    x = np.ascontiguousarray(np.asarray(x, np.float32))
    args = [np.asarray(a, np.float32) for a in
            (w1, w2, b11, b12, b13, b21, b22, b23,
             g1, be1, m1, v1, g2, be2, m2, v2, a1, a2)]
    consts, w1t, w2t = _prep_params(*args)

    if "nc" not in _CACHE:
        _CACHE["nc"] = _build_nc()
    nc = _CACHE["nc"]

    in_maps = []
    for core in range(NCORES):
        in_maps.append({
            "x": np.ascontiguousarray(x[core * HB:(core + 1) * HB]),
            "w1t": w1t,
            "w2t": w2t,
            "consts": consts,
        })
    res = run_bass_kernel_spmd(nc, in_maps, core_ids=list(range(NCORES)))
    out = np.concatenate([r["out"] for r in res.results], axis=0)
    return (out, np.asarray(loss, np.float32))


# revision 79
# speedup vs baseline: 1.0129x; 1.0129x over previous
"""Trainium2 Bass kernel for a ReActNet-style binary BasicBlock.

Full inputs: x [32,256,28,28] f32 + params. Data-parallel over batch across
8 NeuronCores (4 images per core, params replicated, no collectives).

Key algebra (forward pass only):
  _binact(x)  == sign(x)            (STE wrapper vanishes in forward)
  _binweight(w) == mean|w|_o * sign(w)
So each conv is a +-1 x +-1 matmul: exact in fp8e4m3 with fp32 PSUM
accumulation (integer partial sums <= 2304 << 2^24). fp8 enables DoubleRow
matmuls (K=256 per instruction, 2x PE throughput). Per-channel weight scale
and BN affine fold into one multiply-add applied to the PSUM result.

Layout: channels on partitions, 2 channel-blocks of 128 interleaved as the
DoubleRow pair dimension. 3x3 conv = 9 shifted matmuls over a zero-padded
30x30 frame; each matmul streams a contiguous 418-column window (14 padded
rows) and the 392 valid pixels are extracted by the strided PSUM read of the
following fused (s*psum + residual) op.
"""

import numpy as np
import ml_dtypes

import concourse.bacc as bacc
import concourse.mybir as mybir
from concourse.tile import TileContext
from concourse.bass_utils import run_bass_kernel_spmd

F32 = mybir.dt.float32
FP8 = mybir.dt.float8e4
AF = mybir.ActivationFunctionType
OP = mybir.AluOpType
DR = mybir.MatmulPerfMode.DoubleRow

NCORES = 8
P = 128
C = 256
HB = 4            # images per core
H = W = 28
HP = WP = 30      # padded frame
NPIX = HB * H * W    # 3136
IMG = H * W          # 784
CH = IMG // 2        # 392-pixel chunk (half image) per PSUM bank
SEAM = 13 * WP + W   # 418: contiguous window covering 14 padded rows

# consts layout: [128, 2*10] -> per-channel scalars, one column each per block.
# J_SG2B: threshold for sign2 computed directly from u_pre — sign(prelu(t)+c2)
# is monotone in t, so it equals [u_pre + (t1b - t*) >= 0] with t* = prelu^-1(-c2)
# J_NB11 (= -b11): threshold form of sign1 for the DVE fast path at startup
J_B11, J_S1, J_T1B, J_A1, J_SG2B, J_S2, J_T2B, J_A2, J_B23, J_NB11 = range(10)
NCN = 10

_CACHE = {}

X_SPLIT = False      # image-0 load in two row bands
DVE_SIGN1 = False    # image-0 block-1 sign on DVE + per-block x0 DMA
LAST_ADD_DVE = False  # final-group +b23 on DVE instead of GpSimd
GROUPS_CFG = [(0,), (1, 2), (3,)]  # images per weight-reuse group

# emission schedule: ("c1", grp, ob) | ("c2", grp) | ("e1"/"e2", grp, fine)
SCHEDULE = [
    ("c1", 0, 0), ("c1", 0, 1), ("e1", 0, False),
    ("c1", 1, 0, True), ("c2", 0), ("e2", 0, False),
    ("c1", 1, 1, True), ("e1", 1, False),
    ("c1", 2, 0), ("c2", 1), ("e2", 1, False),
    ("c1", 2, 1, True), ("e1", 2, True),
    ("c2", 2), ("e2", 2, True),
]


def _build_nc():
    nc = bacc.Bacc(None, target_bir_lowering=False, debug=False)

    x_t = nc.dram_tensor("x", [HB, C, H, W], F32, kind="ExternalInput")
    w1_t = nc.dram_tensor("w1t", [P, 9 * 2 * C], FP8, kind="ExternalInput")
    w2_t = nc.dram_tensor("w2t", [P, 2 * C], FP8, kind="ExternalInput")
    c_t = nc.dram_tensor("consts", [P, 2 * NCN], F32, kind="ExternalInput")
    out_t = nc.dram_tensor("out", [HB, C, H, W], F32, kind="ExternalOutput")

    with TileContext(nc) as tc:
        with tc.tile_pool(name="main", bufs=1) as pool, \
             tc.tile_pool(name="ps", bufs=1, space="PSUM") as psp:

            def ptile(nm, shape, dt):
                return pool.tile(shape, dt, name=nm, tag=nm)

            c_sb = ptile("c_sb", [P, 2 * NCN], F32)
            x_sb = ptile("x_sb", [P, 2 * NPIX], F32)  # channel-block major
            pad_sb = ptile("pad_sb", [P, 2 * HB * HP * WP], FP8)
            w1_sb = ptile("w1_sb", [P, 9 * 2 * C], FP8)
            w2_sb = ptile("w2_sb", [P, 2 * C], FP8)
            sg2_sb = ptile("sg2_sb", [P, 2 * NPIX], FP8)
            u_pre = [ptile(f"u_pre{i}", [P, NPIX], F32) for i in range(2)]
            p1 = [ptile(f"p1_{i}", [P, NPIX], F32) for i in range(2)]
            v_pre = [ptile(f"v_pre{i}", [P, NPIX], F32) for i in range(2)]
            p2 = [ptile(f"p2_{i}", [P, NPIX], F32) for i in range(2)]

            def cc(blk, j):  # per-channel scalar AP [128,1]
                return c_sb[:, blk * NCN + j:blk * NCN + j + 1]

            def xsl(blk, sl):  # x slice [128, n] for channel block blk
                return x_sb[:, blk * NPIX + sl.start:blk * NPIX + sl.stop]

            # ---- loads, ordered so image 0 can start ASAP (consts ride the
            # scalar engine's HWDGE stream, off the critical sync queue) ----
            nc.scalar.dma_start(out=c_sb, in_=c_t[:])
            xv4 = x_sb.rearrange("p (j b q) -> p j b q", j=2, b=HB)
            xrows0 = x_sb.rearrange("p (j b h w) -> p j b h w",
                                    j=2, b=HB, h=H, w=W)
            # image 0 in two row-bands so Sign (and the first matmuls) can
            # start before the whole image lands
            x0_in = x_t[0].rearrange("(j p) h w -> p j h w", p=P)
            if X_SPLIT:
                nc.sync.dma_start(out=xrows0[:, :, 0, 0:15],
                                  in_=x0_in[:, :, 0:15])
                nc.sync.dma_start(out=xrows0[:, :, 0, 15:H],
                                  in_=x0_in[:, :, 15:H])
            else:
                # per channel block, so block 0's Sign starts ~1us before the
                # second half of image 0 lands
                for j in range(2):
                    nc.sync.dma_start(
                        out=xv4[:, j, 0],
                        in_=x_t[0, j * P:(j + 1) * P].rearrange(
                            "c h w -> c (h w)"))
            # first three kernel positions land first: the opening matmuls
            # only need w1v[:, 0..2] while the rest of the weights stream in
            nc.sync.dma_start(out=w1_sb[:, 0:3 * 2 * C], in_=w1_t[:, 0:3 * 2 * C])
            nc.sync.dma_start(out=w1_sb[:, 3 * 2 * C:], in_=w1_t[:, 3 * 2 * C:])
            for j in range(2):
                nc.sync.dma_start(
                    out=xv4[:, j, 1:HB],
                    in_=x_t[1:HB, j * P:(j + 1) * P].rearrange(
                        "b c h w -> c b (h w)"))
            nc.sync.dma_start(out=w2_sb, in_=w2_t[:])

            # warm the ACT function-table set (Copy shares the set with
            # Sign/Prelu) so the ~2.7us table load overlaps the input DMAs
            dummy = ptile("dummy", [P, 8], F32)
            nc.vector.memset(dummy[:, 0:4], 0.0)
            nc.scalar.activation(dummy[:, 4:8], dummy[:, 0:4], AF.Copy)

            pad5 = pad_sb.rearrange("p (j b h w) -> p j b h w",
                                    j=2, b=HB, h=HP, w=WP)
            pad_j = pad_sb.rearrange("p (j q) -> p j q", j=2)
            w1v = w1_sb.rearrange("p (k j o) -> p k j o", k=9, j=2)
            w2v = w2_sb.rearrange("p (j o) -> p j o", j=2)
            sg2v = sg2_sb.rearrange("p (j q) -> p j q", j=2)

            # zero only the pad borders, per image (interiors are overwritten
            # by Sign): top+bottom rows, then left+right columns
            for b in range(HB):
                for i in range(2):
                    nc.vector.memset(pad5[:, i, b, 0:HP:HP - 1, :], 0.0)
                    nc.vector.memset(pad5[:, i, b, 1:HP - 1, 0:WP:WP - 1], 0.0)

            # sgn1 = Sign(x + b11) written into padded interior, fp8.
            # Image 0 in two row-bands matching its split DMA.
            if X_SPLIT:
                for i in range(2):
                    nc.scalar.activation(
                        pad5[:, i, 0, 1:16, 1:1 + W], xrows0[:, i, 0, 0:15],
                        AF.Sign, bias=cc(i, J_B11))
                for i in range(2):
                    nc.scalar.activation(
                        pad5[:, i, 0, 16:1 + H, 1:1 + W], xrows0[:, i, 0, 15:H],
                        AF.Sign, bias=cc(i, J_B11))
            if not X_SPLIT and DVE_SIGN1:
                # image 0, block 0 on ACT; block 1 on DVE as
                # 2*[x >= -b11] - 1 so both blocks sign concurrently
                nc.scalar.activation(
                    pad5[:, 0, 0, 1:1 + H, 1:1 + W],
                    xsl(0, slice(0, IMG)).rearrange("p (h w) -> p h w", h=H),
                    AF.Sign, bias=cc(0, J_B11))
                sgntmp = ptile("sgntmp", [P, IMG], F32)
                nc.vector.tensor_scalar(sgntmp, xsl(1, slice(0, IMG)),
                                        cc(1, J_NB11), None, op0=OP.is_ge)
                nc.vector.tensor_scalar(
                    pad5[:, 1, 0, 1:1 + H, 1:1 + W],
                    sgntmp.rearrange("p (h w) -> p h w", h=H),
                    2.0, -1.0, op0=OP.mult, op1=OP.add)
            b0_range = range(1, HB) if (X_SPLIT or DVE_SIGN1) else range(HB)
            for b in b0_range:
                for i in range(2):
                    nc.scalar.activation(
                        pad5[:, i, b, 1:1 + H, 1:1 + W],
                        xsl(i, slice(b * IMG, (b + 1) * IMG)).rearrange(
                            "p (h w) -> p h w", h=H),
                        AF.Sign, bias=cc(i, J_B11))

            # chunk groups sharing stationary weights: singleton first/last
            # groups shorten the kernel head (image 0 prep only) and tail
            GROUPS = GROUPS_CFG

            def grp_chunks(grp):
                return [(b, half) for b in GROUPS[grp] for half in range(2)]

            def chunk_sl(b, half):
                return slice(b * IMG + half * CH, b * IMG + half * CH + CH)

            def r14x28(ap_flat_392):
                return ap_flat_392.rearrange("p (r c) -> p r c", c=W)

            def conv1_group(grp, ob, seq=False):
                chunks = grp_chunks(grp)
                # ps_0/ps_1 double-buffer across groups; ps_2/ps_3 are only
                # used by the 4-chunk middle groups (single-buffered) so two
                # banks stay free for conv2's dedicated tiles
                pss = [psp.tile([P, 420], F32, name=f"ps_{ci}",
                                tag=f"ps_{ci}", bufs=(2 if ci < 2 else 1))
                       for ci in range(len(chunks))]
                # seq=True: finish chunks one at a time (extra LDWEIGHTS, but
                # the per-chunk epilogue chain starts a weight-pass earlier)
                chunk_sets = ([[c] for c in chunks] if seq else [chunks])
                for cset in chunk_sets:
                    for k in range(9):
                        ky, kx = divmod(k, 3)
                        w = w1v[:, k, :, ob * P:(ob + 1) * P]
                        for b, half in cset:
                            ci = chunks.index((b, half))
                            off = b * HP * WP + (half * (H // 2) + ky) * WP + kx
                            nc.tensor.matmul(
                                pss[ci][:, 0:SEAM], w,
                                pad_j[:, :, off:off + SEAM],
                                start=(k == 0), stop=(k == 8), perf_mode=DR)
                for ci, (b, half) in enumerate(chunks):
                    # u_pre = s1 * psum + x  (valid 392 of the 418 columns)
                    psv = pss[ci].rearrange("p (r c) -> p r c", c=WP)[:, 0:14, 0:W]
                    sl = chunk_sl(b, half)
                    nc.vector.scalar_tensor_tensor(
                        r14x28(u_pre[ob][:, sl]), psv, cc(ob, J_S1),
                        r14x28(xsl(ob, sl)), op0=OP.mult, op1=OP.add)

            def branch1_ew(grp, fine):
                # branch-2 activation as a {0,1} indicator on GpSimd, straight
                # from u_pre (prelu1 is off this path): conv2(2u-1) is
                # recovered via s2'=2*s2 and the host-folded rowsum in t2b'.
                # Always chunk-grained: it gates the next conv2 group.
                for b, half in grp_chunks(grp):
                    sl = chunk_sl(b, half)
                    for ob in range(2):
                        nc.gpsimd.tensor_scalar(
                            sg2v[:, ob, sl], u_pre[ob][:, sl],
                            cc(ob, J_SG2B), 0.0, op0=OP.add, op1=OP.is_ge)
                # p1 = prelu(u_pre + (bn1_const + b12), a1): residual only
                if fine:
                    pieces = [chunk_sl(b, half) for b, half in grp_chunks(grp)]
                else:
                    pieces = [slice(b * IMG, (b + 1) * IMG) for b in GROUPS[grp]]
                for sl in pieces:
                    for ob in range(2):
                        nc.scalar.activation(p1[ob][:, sl], u_pre[ob][:, sl],
                                             AF.Prelu, bias=cc(ob, J_T1B),
                                             alpha=cc(ob, J_A1))

            def conv2_group(grp):
                chunks = grp_chunks(grp)
                for ob in range(2):
                    w = w2v[:, :, ob * P:(ob + 1) * P]
                    pss = [psp.tile([P, 420], F32, name=f"cps_{ci % 2}",
                                    tag=f"cps_{ci % 2}", bufs=1)
                           for ci in range(len(chunks))]
                    for ci, (b, half) in enumerate(chunks):
                        nc.tensor.matmul(
                            pss[ci][:, 0:CH], w, sg2v[:, :, chunk_sl(b, half)],
                            start=True, stop=True, perf_mode=DR)
                    for ci, (b, half) in enumerate(chunks):
                        # v_pre = s2 * psum + p1
                        sl = chunk_sl(b, half)
                        nc.vector.scalar_tensor_tensor(
                            v_pre[ob][:, sl], pss[ci][:, 0:CH], cc(ob, J_S2),
                            p1[ob][:, sl], op0=OP.mult, op1=OP.add)

            def branch2_ew(grp, fine):
                pieces = (grp_chunks(grp) if fine
                          else [(b, None) for b in GROUPS[grp]])
                for b, half in pieces:
                    sl = chunk_sl(b, half) if half is not None else \
                        slice(b * IMG, (b + 1) * IMG)
                    for ob in range(2):
                        # p2 = prelu(v_pre + (bn2_const + b22 + b13), a2)
                        nc.scalar.activation(
                            p2[ob][:, sl], v_pre[ob][:, sl],
                            AF.Prelu, bias=cc(ob, J_T2B),
                            alpha=cc(ob, J_A2))
                        # out = p2 + b23, staged into x_sb (x is dead by now)
                        eng = (nc.vector if LAST_ADD_DVE
                               and grp == len(GROUPS) - 1 else nc.gpsimd)
                        eng.tensor_scalar_add(xsl(ob, sl),
                                              p2[ob][:, sl], cc(ob, J_B23))
                    hw0 = 0 if half is None else half * (H // 2)
                    nrows = H if half is None else H // 2
                    if half is None:
                        # one DMA per image covering both channel blocks
                        xq = x_sb.rearrange("p (j q) -> p j q", j=2)
                        nc.sync.dma_start(
                            out=out_t[b, :, hw0:hw0 + nrows].rearrange(
                                "(j p) h w -> p j (h w)", p=P),
                            in_=xq[:, :, sl.start:sl.stop])
                    else:
                        # tail pieces ship per channel block, as soon as ready
                        for ob in range(2):
                            nc.sync.dma_start(
                                out=out_t[b, ob * P:(ob + 1) * P,
                                          hw0:hw0 + nrows].rearrange(
                                    "c h w -> c (h w)"),
                                in_=xsl(ob, sl))

            # schedule: each conv2 group is sandwiched inside the next conv1
            # group so the PE never waits on the elementwise chain; the final
            # group (image 3 only) runs fine-grained (392-px pieces) to
            # shorten the kernel tail.
            ops = {
                "c1": conv1_group,
                "c2": conv2_group,
                "e1": branch1_ew,
                "e2": branch2_ew,
            }
            for step in SCHEDULE:
                ops[step[0]](*step[1:])

    _dedup_ldweights(nc)
    nc.compile()
    return nc


def _dedup_ldweights(nc):
    """Drop InstLdweights that reload the exact weights already resident in
    the PE array (weight-reuse groups emit one matmul per pixel chunk, and the
    tile lowering pairs every matmul with its own Ldweights). Only wait-free,
    update-free duplicates are removed, so semaphore semantics are untouched."""
    pe = nc.tensor.engine
    removed = 0
    for blk in nc.m.functions[0].blocks:
        last_sig = None
        keep = []
        for ins in blk.instructions:
            if ins.engine == pe:
                t = type(ins).__name__
                if t == "InstLdweights":
                    sig = repr(ins.ins[0])
                    if (sig == last_sig and not ins.has_wait()
                            and not ins.has_update()):
                        removed += 1
                        continue
                    last_sig = sig
                elif t not in ("InstMatmult", "InstEventSemaphore"):
                    last_sig = None
            keep.append(ins)
        if removed:
            while len(blk.instructions):
                blk.instructions.pop()
            for ins in keep:
                blk.instructions.append(ins)
    return removed


def _prep_params(w1, w2, b11, b12, b13, b21, b22, b23,
                 g1, be1, m1, v1, g2, be2, m2, v2, a1, a2):
    f = np.float32
    eps = f(1e-5)
    scale1 = np.mean(np.abs(w1), axis=(1, 2, 3), dtype=np.float32)
    scale2 = np.mean(np.abs(w2), axis=(1, 2, 3), dtype=np.float32)
    inv1 = (g1 / np.sqrt(v1 + eps)).astype(f)
    inv2 = (g2 / np.sqrt(v2 + eps)).astype(f)
    s1 = (scale1 * inv1).astype(f)
    s2 = (scale2 * inv2).astype(f)
    t1b = ((be1 - m1 * inv1) + b12).astype(f)
    c2 = (b13 + b21).astype(f)
    t2b = (((be2 - m2 * inv2) + b22) + b13).astype(f)
    # branch-2 activations are sent as indicators u in {0,1}:
    # conv2(2u-1)[o] = 2*conv2(u)[o] - rowsum(sign(w2))[o]
    rs2 = np.sign(w2[:, :, 0, 0]).astype(f).sum(axis=1, dtype=np.float32)
    s2_eff = (2.0 * s2).astype(f)
    t2b_eff = (t2b - s2 * rs2).astype(f)
    # sign2 threshold straight from u_pre: sign(prelu(t)+c2) = [t >= t*],
    # t* = -c2 (c2<=0) else -c2/a1  (prelu is monotone for a1>0)
    a1_safe = np.where(a1 != 0, a1, f(1.0)).astype(f)
    t_star = np.where(c2 <= 0, -c2, -(c2 / a1_safe)).astype(f)
    sg2b = (t1b - t_star).astype(f)

    consts = np.stack([b11, s1, t1b, a1, sg2b, s2_eff, t2b_eff, a2, b23,
                       -b11], axis=1).astype(f).reshape(2, P, NCN)
    consts = np.ascontiguousarray(consts.transpose(1, 0, 2).reshape(P, 2 * NCN))

    fp8 = ml_dtypes.float8_e4m3
    # w1t[p, k, j, o] = sign(w1)[o, j*128+p, ky, kx]
    w1t = np.sign(w1).astype(np.float32).transpose(1, 2, 3, 0)  # [I,ky,kx,O]
    w1t = w1t.reshape(2, P, 9, C).transpose(1, 2, 0, 3)          # [p,k,j,o]
    w1t = np.ascontiguousarray(w1t.reshape(P, 9 * 2 * C)).astype(fp8)
    # w2t[p, j, o] = sign(w2)[o, j*128+p]
    w2t = np.sign(w2[:, :, 0, 0]).astype(np.float32).T.reshape(2, P, C)
    w2t = np.ascontiguousarray(w2t.transpose(1, 0, 2).reshape(P, 2 * C)).astype(fp8)
    return consts, w1t, w2t


def kernel(x, loss, b11, b12, b13, b21, b22, b23, w1, w2,
           g1, be1, m1, v1, g2, be2, m2, v2, a1, a2):
    x = np.ascontiguousarray(np.asarray(x, np.float32))
    args = [np.asarray(a, np.float32) for a in
            (w1, w2, b11, b12, b13, b21, b22, b23,
             g1, be1, m1, v1, g2, be2, m2, v2, a1, a2)]
    consts, w1t, w2t = _prep_params(*args)

    if "nc" not in _CACHE:
        _CACHE["nc"] = _build_nc()
    nc = _CACHE["nc"]

    in_maps = []
    for core in range(NCORES):
        in_maps.append({
            "x": np.ascontiguousarray(x[core * HB:(core + 1) * HB]),
            "w1t": w1t,
            "w2t": w2t,
            "consts": consts,
        })
    res = run_bass_kernel_spmd(nc, in_maps, core_ids=list(range(NCORES)))
    out = np.concatenate([r["out"] for r in res.results], axis=0)
    return (out, np.asarray(loss, np.float32))


# revision 80
# speedup vs baseline: 1.0156x; 1.0027x over previous
"""Trainium2 Bass kernel for a ReActNet-style binary BasicBlock.

Full inputs: x [32,256,28,28] f32 + params. Data-parallel over batch across
8 NeuronCores (4 images per core, params replicated, no collectives).

Key algebra (forward pass only):
  _binact(x)  == sign(x)            (STE wrapper vanishes in forward)
  _binweight(w) == mean|w|_o * sign(w)
So each conv is a +-1 x +-1 matmul: exact in fp8e4m3 with fp32 PSUM
accumulation (integer partial sums <= 2304 << 2^24). fp8 enables DoubleRow
matmuls (K=256 per instruction, 2x PE throughput). Per-channel weight scale
and BN affine fold into one multiply-add applied to the PSUM result.

Layout: channels on partitions, 2 channel-blocks of 128 interleaved as the
DoubleRow pair dimension. 3x3 conv = 9 shifted matmuls over a zero-padded
30x30 frame; each matmul streams a contiguous 418-column window (14 padded
rows) and the 392 valid pixels are extracted by the strided PSUM read of the
following fused (s*psum + residual) op.
"""

import numpy as np
import ml_dtypes

import concourse.bacc as bacc
import concourse.mybir as mybir
from concourse.tile import TileContext
from concourse.bass_utils import run_bass_kernel_spmd

F32 = mybir.dt.float32
FP8 = mybir.dt.float8e4
AF = mybir.ActivationFunctionType
OP = mybir.AluOpType
DR = mybir.MatmulPerfMode.DoubleRow

NCORES = 8
P = 128
C = 256
HB = 4            # images per core
H = W = 28
HP = WP = 30      # padded frame
NPIX = HB * H * W    # 3136
IMG = H * W          # 784
CH = IMG // 2        # 392-pixel chunk (half image) per PSUM bank
SEAM = 13 * WP + W   # 418: contiguous window covering 14 padded rows

# consts layout: [128, 2*10] -> per-channel scalars, one column each per block.
# J_SG2B: threshold for sign2 computed directly from u_pre — sign(prelu(t)+c2)
# is monotone in t, so it equals [u_pre + (t1b - t*) >= 0] with t* = prelu^-1(-c2)
# J_NB11 (= -b11): threshold form of sign1 for the DVE fast path at startup
J_B11, J_S1, J_T1B, J_A1, J_SG2B, J_S2, J_T2B, J_A2, J_B23, J_NB11 = range(10)
NCN = 10

_CACHE = {}

X_SPLIT = False      # image-0 load in two row bands
DVE_SIGN1 = False    # image-0 block-1 sign on DVE + per-block x0 DMA
LAST_ADD_DVE = False  # final-group +b23 on DVE instead of GpSimd
GROUPS_CFG = [(0,), (1, 2), (3,)]  # images per weight-reuse group

# emission schedule: ("c1", grp, ob) | ("c2", grp) | ("e1"/"e2", grp, fine)
SCHEDULE = [
    ("c1", 0, 0), ("c1", 0, 1), ("e1", 0, False),
    ("c1", 1, 0, True), ("c2", 0), ("e2", 0, False),
    ("c1", 1, 1, True), ("e1", 1, False),
    ("c1", 2, 0), ("c2", 1), ("e2", 1, True),
    ("c1", 2, 1, True), ("e1", 2, True),
    ("c2", 2), ("e2", 2, True),
]


def _build_nc():
    nc = bacc.Bacc(None, target_bir_lowering=False, debug=False)

    x_t = nc.dram_tensor("x", [HB, C, H, W], F32, kind="ExternalInput")
    w1_t = nc.dram_tensor("w1t", [P, 9 * 2 * C], FP8, kind="ExternalInput")
    w2_t = nc.dram_tensor("w2t", [P, 2 * C], FP8, kind="ExternalInput")
    c_t = nc.dram_tensor("consts", [P, 2 * NCN], F32, kind="ExternalInput")
    out_t = nc.dram_tensor("out", [HB, C, H, W], F32, kind="ExternalOutput")

    with TileContext(nc) as tc:
        with tc.tile_pool(name="main", bufs=1) as pool, \
             tc.tile_pool(name="ps", bufs=1, space="PSUM") as psp:

            def ptile(nm, shape, dt):
                return pool.tile(shape, dt, name=nm, tag=nm)

            c_sb = ptile("c_sb", [P, 2 * NCN], F32)
            x_sb = ptile("x_sb", [P, 2 * NPIX], F32)  # channel-block major
            pad_sb = ptile("pad_sb", [P, 2 * HB * HP * WP], FP8)
            w1_sb = ptile("w1_sb", [P, 9 * 2 * C], FP8)
            w2_sb = ptile("w2_sb", [P, 2 * C], FP8)
            sg2_sb = ptile("sg2_sb", [P, 2 * NPIX], FP8)
            u_pre = [ptile(f"u_pre{i}", [P, NPIX], F32) for i in range(2)]
            p1 = [ptile(f"p1_{i}", [P, NPIX], F32) for i in range(2)]
            v_pre = [ptile(f"v_pre{i}", [P, NPIX], F32) for i in range(2)]
            p2 = [ptile(f"p2_{i}", [P, NPIX], F32) for i in range(2)]

            def cc(blk, j):  # per-channel scalar AP [128,1]
                return c_sb[:, blk * NCN + j:blk * NCN + j + 1]

            def xsl(blk, sl):  # x slice [128, n] for channel block blk
                return x_sb[:, blk * NPIX + sl.start:blk * NPIX + sl.stop]

            # ---- loads, ordered so image 0 can start ASAP (consts ride the
            # scalar engine's HWDGE stream, off the critical sync queue) ----
            nc.scalar.dma_start(out=c_sb, in_=c_t[:])
            xv4 = x_sb.rearrange("p (j b q) -> p j b q", j=2, b=HB)
            xrows0 = x_sb.rearrange("p (j b h w) -> p j b h w",
                                    j=2, b=HB, h=H, w=W)
            # image 0 in two row-bands so Sign (and the first matmuls) can
            # start before the whole image lands
            x0_in = x_t[0].rearrange("(j p) h w -> p j h w", p=P)
            if X_SPLIT:
                nc.sync.dma_start(out=xrows0[:, :, 0, 0:15],
                                  in_=x0_in[:, :, 0:15])
                nc.sync.dma_start(out=xrows0[:, :, 0, 15:H],
                                  in_=x0_in[:, :, 15:H])
            else:
                # per channel block, so block 0's Sign starts ~1us before the
                # second half of image 0 lands
                for j in range(2):
                    nc.sync.dma_start(
                        out=xv4[:, j, 0],
                        in_=x_t[0, j * P:(j + 1) * P].rearrange(
                            "c h w -> c (h w)"))
            # first three kernel positions land first: the opening matmuls
            # only need w1v[:, 0..2] while the rest of the weights stream in
            nc.sync.dma_start(out=w1_sb[:, 0:3 * 2 * C], in_=w1_t[:, 0:3 * 2 * C])
            nc.sync.dma_start(out=w1_sb[:, 3 * 2 * C:], in_=w1_t[:, 3 * 2 * C:])
            for j in range(2):
                nc.sync.dma_start(
                    out=xv4[:, j, 1:HB],
                    in_=x_t[1:HB, j * P:(j + 1) * P].rearrange(
                        "b c h w -> c b (h w)"))
            nc.sync.dma_start(out=w2_sb, in_=w2_t[:])

            # warm the ACT function-table set (Copy shares the set with
            # Sign/Prelu) so the ~2.7us table load overlaps the input DMAs
            dummy = ptile("dummy", [P, 8], F32)
            nc.vector.memset(dummy[:, 0:4], 0.0)
            nc.scalar.activation(dummy[:, 4:8], dummy[:, 0:4], AF.Copy)

            pad5 = pad_sb.rearrange("p (j b h w) -> p j b h w",
                                    j=2, b=HB, h=HP, w=WP)
            pad_j = pad_sb.rearrange("p (j q) -> p j q", j=2)
            w1v = w1_sb.rearrange("p (k j o) -> p k j o", k=9, j=2)
            w2v = w2_sb.rearrange("p (j o) -> p j o", j=2)
            sg2v = sg2_sb.rearrange("p (j q) -> p j q", j=2)

            # zero only the pad borders, per image (interiors are overwritten
            # by Sign): top+bottom rows, then left+right columns
            for b in range(HB):
                for i in range(2):
                    nc.vector.memset(pad5[:, i, b, 0:HP:HP - 1, :], 0.0)
                    nc.vector.memset(pad5[:, i, b, 1:HP - 1, 0:WP:WP - 1], 0.0)

            # sgn1 = Sign(x + b11) written into padded interior, fp8.
            # Image 0 in two row-bands matching its split DMA.
            if X_SPLIT:
                for i in range(2):
                    nc.scalar.activation(
                        pad5[:, i, 0, 1:16, 1:1 + W], xrows0[:, i, 0, 0:15],
                        AF.Sign, bias=cc(i, J_B11))
                for i in range(2):
                    nc.scalar.activation(
                        pad5[:, i, 0, 16:1 + H, 1:1 + W], xrows0[:, i, 0, 15:H],
                        AF.Sign, bias=cc(i, J_B11))
            if not X_SPLIT and DVE_SIGN1:
                # image 0, block 0 on ACT; block 1 on DVE as
                # 2*[x >= -b11] - 1 so both blocks sign concurrently
                nc.scalar.activation(
                    pad5[:, 0, 0, 1:1 + H, 1:1 + W],
                    xsl(0, slice(0, IMG)).rearrange("p (h w) -> p h w", h=H),
                    AF.Sign, bias=cc(0, J_B11))
                sgntmp = ptile("sgntmp", [P, IMG], F32)
                nc.vector.tensor_scalar(sgntmp, xsl(1, slice(0, IMG)),
                                        cc(1, J_NB11), None, op0=OP.is_ge)
                nc.vector.tensor_scalar(
                    pad5[:, 1, 0, 1:1 + H, 1:1 + W],
                    sgntmp.rearrange("p (h w) -> p h w", h=H),
                    2.0, -1.0, op0=OP.mult, op1=OP.add)
            b0_range = range(1, HB) if (X_SPLIT or DVE_SIGN1) else range(HB)
            for b in b0_range:
                for i in range(2):
                    nc.scalar.activation(
                        pad5[:, i, b, 1:1 + H, 1:1 + W],
                        xsl(i, slice(b * IMG, (b + 1) * IMG)).rearrange(
                            "p (h w) -> p h w", h=H),
                        AF.Sign, bias=cc(i, J_B11))

            # chunk groups sharing stationary weights: singleton first/last
            # groups shorten the kernel head (image 0 prep only) and tail
            GROUPS = GROUPS_CFG

            def grp_chunks(grp):
                return [(b, half) for b in GROUPS[grp] for half in range(2)]

            def chunk_sl(b, half):
                return slice(b * IMG + half * CH, b * IMG + half * CH + CH)

            def r14x28(ap_flat_392):
                return ap_flat_392.rearrange("p (r c) -> p r c", c=W)

            def conv1_group(grp, ob, seq=False):
                chunks = grp_chunks(grp)
                # ps_0/ps_1 double-buffer across groups; ps_2/ps_3 are only
                # used by the 4-chunk middle groups (single-buffered) so two
                # banks stay free for conv2's dedicated tiles
                pss = [psp.tile([P, 420], F32, name=f"ps_{ci}",
                                tag=f"ps_{ci}", bufs=(2 if ci < 2 else 1))
                       for ci in range(len(chunks))]
                # seq=True: finish chunks one at a time (extra LDWEIGHTS, but
                # the per-chunk epilogue chain starts a weight-pass earlier)
                chunk_sets = ([[c] for c in chunks] if seq else [chunks])
                for cset in chunk_sets:
                    for k in range(9):
                        ky, kx = divmod(k, 3)
                        w = w1v[:, k, :, ob * P:(ob + 1) * P]
                        for b, half in cset:
                            ci = chunks.index((b, half))
                            off = b * HP * WP + (half * (H // 2) + ky) * WP + kx
                            nc.tensor.matmul(
                                pss[ci][:, 0:SEAM], w,
                                pad_j[:, :, off:off + SEAM],
                                start=(k == 0), stop=(k == 8), perf_mode=DR)
                for ci, (b, half) in enumerate(chunks):
                    # u_pre = s1 * psum + x  (valid 392 of the 418 columns)
                    psv = pss[ci].rearrange("p (r c) -> p r c", c=WP)[:, 0:14, 0:W]
                    sl = chunk_sl(b, half)
                    nc.vector.scalar_tensor_tensor(
                        r14x28(u_pre[ob][:, sl]), psv, cc(ob, J_S1),
                        r14x28(xsl(ob, sl)), op0=OP.mult, op1=OP.add)

            def branch1_ew(grp, fine):
                # branch-2 activation as a {0,1} indicator on GpSimd, straight
                # from u_pre (prelu1 is off this path): conv2(2u-1) is
                # recovered via s2'=2*s2 and the host-folded rowsum in t2b'.
                # Always chunk-grained: it gates the next conv2 group.
                for b, half in grp_chunks(grp):
                    sl = chunk_sl(b, half)
                    for ob in range(2):
                        nc.gpsimd.tensor_scalar(
                            sg2v[:, ob, sl], u_pre[ob][:, sl],
                            cc(ob, J_SG2B), 0.0, op0=OP.add, op1=OP.is_ge)
                # p1 = prelu(u_pre + (bn1_const + b12), a1): residual only
                if fine:
                    pieces = [chunk_sl(b, half) for b, half in grp_chunks(grp)]
                else:
                    pieces = [slice(b * IMG, (b + 1) * IMG) for b in GROUPS[grp]]
                for sl in pieces:
                    for ob in range(2):
                        nc.scalar.activation(p1[ob][:, sl], u_pre[ob][:, sl],
                                             AF.Prelu, bias=cc(ob, J_T1B),
                                             alpha=cc(ob, J_A1))

            def conv2_group(grp):
                chunks = grp_chunks(grp)
                for ob in range(2):
                    w = w2v[:, :, ob * P:(ob + 1) * P]
                    pss = [psp.tile([P, 420], F32, name=f"cps_{ci % 2}",
                                    tag=f"cps_{ci % 2}", bufs=1)
                           for ci in range(len(chunks))]
                    for ci, (b, half) in enumerate(chunks):
                        nc.tensor.matmul(
                            pss[ci][:, 0:CH], w, sg2v[:, :, chunk_sl(b, half)],
                            start=True, stop=True, perf_mode=DR)
                    for ci, (b, half) in enumerate(chunks):
                        # v_pre = s2 * psum + p1
                        sl = chunk_sl(b, half)
                        nc.vector.scalar_tensor_tensor(
                            v_pre[ob][:, sl], pss[ci][:, 0:CH], cc(ob, J_S2),
                            p1[ob][:, sl], op0=OP.mult, op1=OP.add)

            def branch2_ew(grp, fine):
                pieces = (grp_chunks(grp) if fine
                          else [(b, None) for b in GROUPS[grp]])
                for b, half in pieces:
                    sl = chunk_sl(b, half) if half is not None else \
                        slice(b * IMG, (b + 1) * IMG)
                    for ob in range(2):
                        # p2 = prelu(v_pre + (bn2_const + b22 + b13), a2)
                        nc.scalar.activation(
                            p2[ob][:, sl], v_pre[ob][:, sl],
                            AF.Prelu, bias=cc(ob, J_T2B),
                            alpha=cc(ob, J_A2))
                        # out = p2 + b23, staged into x_sb (x is dead by now)
                        eng = (nc.vector if LAST_ADD_DVE
                               and grp == len(GROUPS) - 1 else nc.gpsimd)
                        eng.tensor_scalar_add(xsl(ob, sl),
                                              p2[ob][:, sl], cc(ob, J_B23))
                    hw0 = 0 if half is None else half * (H // 2)
                    nrows = H if half is None else H // 2
                    if half is None:
                        # one DMA per image covering both channel blocks
                        xq = x_sb.rearrange("p (j q) -> p j q", j=2)
                        nc.sync.dma_start(
                            out=out_t[b, :, hw0:hw0 + nrows].rearrange(
                                "(j p) h w -> p j (h w)", p=P),
                            in_=xq[:, :, sl.start:sl.stop])
                    else:
                        # tail pieces ship per channel block, as soon as ready
                        for ob in range(2):
                            nc.sync.dma_start(
                                out=out_t[b, ob * P:(ob + 1) * P,
                                          hw0:hw0 + nrows].rearrange(
                                    "c h w -> c (h w)"),
                                in_=xsl(ob, sl))

            # schedule: each conv2 group is sandwiched inside the next conv1
            # group so the PE never waits on the elementwise chain; the final
            # group (image 3 only) runs fine-grained (392-px pieces) to
            # shorten the kernel tail.
            ops = {
                "c1": conv1_group,
                "c2": conv2_group,
                "e1": branch1_ew,
                "e2": branch2_ew,
            }
            for step in SCHEDULE:
                ops[step[0]](*step[1:])

    _dedup_ldweights(nc)
    nc.compile()
    return nc


def _dedup_ldweights(nc):
    """Drop InstLdweights that reload the exact weights already resident in
    the PE array (weight-reuse groups emit one matmul per pixel chunk, and the
    tile lowering pairs every matmul with its own Ldweights). Only wait-free,
    update-free duplicates are removed, so semaphore semantics are untouched."""
    pe = nc.tensor.engine
    removed = 0
    for blk in nc.m.functions[0].blocks:
        last_sig = None
        keep = []
        for ins in blk.instructions:
            if ins.engine == pe:
                t = type(ins).__name__
                if t == "InstLdweights":
                    sig = repr(ins.ins[0])
                    if (sig == last_sig and not ins.has_wait()
                            and not ins.has_update()):
                        removed += 1
                        continue
                    last_sig = sig
                elif t not in ("InstMatmult", "InstEventSemaphore"):
                    last_sig = None
            keep.append(ins)
        if removed:
            while len(blk.instructions):
                blk.instructions.pop()
            for ins in keep:
                blk.instructions.append(ins)
    return removed


def _prep_params(w1, w2, b11, b12, b13, b21, b22, b23,
                 g1, be1, m1, v1, g2, be2, m2, v2, a1, a2):
    f = np.float32
    eps = f(1e-5)
    scale1 = np.mean(np.abs(w1), axis=(1, 2, 3), dtype=np.float32)
    scale2 = np.mean(np.abs(w2), axis=(1, 2, 3), dtype=np.float32)
    inv1 = (g1 / np.sqrt(v1 + eps)).astype(f)
    inv2 = (g2 / np.sqrt(v2 + eps)).astype(f)
    s1 = (scale1 * inv1).astype(f)
    s2 = (scale2 * inv2).astype(f)
    t1b = ((be1 - m1 * inv1) + b12).astype(f)
    c2 = (b13 + b21).astype(f)
    t2b = (((be2 - m2 * inv2) + b22) + b13).astype(f)
    # branch-2 activations are sent as indicators u in {0,1}:
    # conv2(2u-1)[o] = 2*conv2(u)[o] - rowsum(sign(w2))[o]
    rs2 = np.sign(w2[:, :, 0, 0]).astype(f).sum(axis=1, dtype=np.float32)
    s2_eff = (2.0 * s2).astype(f)
    t2b_eff = (t2b - s2 * rs2).astype(f)
    # sign2 threshold straight from u_pre: sign(prelu(t)+c2) = [t >= t*],
    # t* = -c2 (c2<=0) else -c2/a1  (prelu is monotone for a1>0)
    a1_safe = np.where(a1 != 0, a1, f(1.0)).astype(f)
    t_star = np.where(c2 <= 0, -c2, -(c2 / a1_safe)).astype(f)
    sg2b = (t1b - t_star).astype(f)

    consts = np.stack([b11, s1, t1b, a1, sg2b, s2_eff, t2b_eff, a2, b23,
                       -b11], axis=1).astype(f).reshape(2, P, NCN)
    consts = np.ascontiguousarray(consts.transpose(1, 0, 2).reshape(P, 2 * NCN))

    fp8 = ml_dtypes.float8_e4m3
    # w1t[p, k, j, o] = sign(w1)[o, j*128+p, ky, kx]
    w1t = np.sign(w1).astype(np.float32).transpose(1, 2, 3, 0)  # [I,ky,kx,O]
    w1t = w1t.reshape(2, P, 9, C).transpose(1, 2, 0, 3)          # [p,k,j,o]
    w1t = np.ascontiguousarray(w1t.reshape(P, 9 * 2 * C)).astype(fp8)
    # w2t[p, j, o] = sign(w2)[o, j*128+p]
    w2t = np.sign(w2[:, :, 0, 0]).astype(np.float32).T.reshape(2, P, C)
    w2t = np.ascontiguousarray(w2t.transpose(1, 0, 2).reshape(P, 2 * C)).astype(fp8)
    return consts, w1t, w2t


def kernel(x, loss, b11, b12, b13, b21, b22, b23, w1, w2,
           g1, be1, m1, v1, g2, be2, m2, v2, a1, a2):
    x = np.ascontiguousarray(np.asarray(x, np.float32))
    args = [np.asarray(a, np.float32) for a in
            (w1, w2, b11, b12, b13, b21, b22, b23,
             g1, be1, m1, v1, g2, be2, m2, v2, a1, a2)]
    consts, w1t, w2t = _prep_params(*args)

    if "nc" not in _CACHE:
        _CACHE["nc"] = _build_nc()
    nc = _CACHE["nc"]

    in_maps = []
    for core in range(NCORES):
        in_maps.append({
            "x": np.ascontiguousarray(x[core * HB:(core + 1) * HB]),
            "w1t": w1t,
            "w2t": w2t,
            "consts": consts,
        })
    res = run_bass_kernel_spmd(nc, in_maps, core_ids=list(range(NCORES)))
    out = np.concatenate([r["out"] for r in res.results], axis=0)
    return (out, np.asarray(loss, np.float32))


# revision 81
# speedup vs baseline: 1.0283x; 1.0125x over previous
"""Trainium2 Bass kernel for a ReActNet-style binary BasicBlock.

Full inputs: x [32,256,28,28] f32 + params. Data-parallel over batch across
8 NeuronCores (4 images per core, params replicated, no collectives).

Key algebra (forward pass only):
  _binact(x)  == sign(x)            (STE wrapper vanishes in forward)
  _binweight(w) == mean|w|_o * sign(w)
So each conv is a +-1 x +-1 matmul: exact in fp8e4m3 with fp32 PSUM
accumulation (integer partial sums <= 2304 << 2^24). fp8 enables DoubleRow
matmuls (K=256 per instruction, 2x PE throughput). Per-channel weight scale
and BN affine fold into one multiply-add applied to the PSUM result.

Layout: channels on partitions, 2 channel-blocks of 128 interleaved as the
DoubleRow pair dimension. 3x3 conv = 9 shifted matmuls over a zero-padded
30x30 frame; each matmul streams a contiguous 418-column window (14 padded
rows) and the 392 valid pixels are extracted by the strided PSUM read of the
following fused (s*psum + residual) op.
"""

import numpy as np
import ml_dtypes

import concourse.bacc as bacc
import concourse.mybir as mybir
from concourse.tile import TileContext
from concourse.bass_utils import run_bass_kernel_spmd

F32 = mybir.dt.float32
FP8 = mybir.dt.float8e4
AF = mybir.ActivationFunctionType
OP = mybir.AluOpType
DR = mybir.MatmulPerfMode.DoubleRow

NCORES = 8
P = 128
C = 256
HB = 4            # images per core
H = W = 28
HP = WP = 30      # padded frame
NPIX = HB * H * W    # 3136
IMG = H * W          # 784
CH = IMG // 2        # 392-pixel chunk (half image) per PSUM bank
SEAM = 13 * WP + W   # 418: contiguous window covering 14 padded rows

# consts layout: [128, 2*10] -> per-channel scalars, one column each per block.
# J_SG2B: threshold for sign2 computed directly from u_pre — sign(prelu(t)+c2)
# is monotone in t, so it equals [u_pre + (t1b - t*) >= 0] with t* = prelu^-1(-c2)
# J_NB11 (= -b11): threshold form of sign1 for the DVE fast path at startup
J_B11, J_S1, J_T1B, J_A1, J_SG2B, J_S2, J_T2B, J_A2, J_B23, J_NB11 = range(10)
NCN = 10

_CACHE = {}

X_SPLIT = False      # image-0 load in two row bands
DVE_SIGN1 = False    # image-0 block-1 sign on DVE + per-block x0 DMA
LAST_ADD_DVE = False  # final-group +b23 on DVE instead of GpSimd
GROUPS_CFG = [(0,), (1, 2), (3,)]  # images per weight-reuse group

# emission schedule: ("c1", grp, ob) | ("c2", grp) | ("e1"/"e2", grp, fine)
SCHEDULE = [
    ("c1", 0, 0, True), ("c1", 0, 1), ("e1", 0, True),
    ("c1", 1, 0, True), ("c2", 0), ("e2", 0, False),
    ("c1", 1, 1, True), ("e1", 1, False),
    ("c1", 2, 0), ("c2", 1), ("e2", 1, True),
    ("c1", 2, 1, True), ("e1", 2, True),
    ("c2", 2), ("e2", 2, True),
]


def _build_nc():
    nc = bacc.Bacc(None, target_bir_lowering=False, debug=False)

    x_t = nc.dram_tensor("x", [HB, C, H, W], F32, kind="ExternalInput")
    w1_t = nc.dram_tensor("w1t", [P, 9 * 2 * C], FP8, kind="ExternalInput")
    w2_t = nc.dram_tensor("w2t", [P, 2 * C], FP8, kind="ExternalInput")
    c_t = nc.dram_tensor("consts", [P, 2 * NCN], F32, kind="ExternalInput")
    out_t = nc.dram_tensor("out", [HB, C, H, W], F32, kind="ExternalOutput")

    with TileContext(nc) as tc:
        with tc.tile_pool(name="main", bufs=1) as pool, \
             tc.tile_pool(name="ps", bufs=1, space="PSUM") as psp:

            def ptile(nm, shape, dt):
                return pool.tile(shape, dt, name=nm, tag=nm)

            c_sb = ptile("c_sb", [P, 2 * NCN], F32)
            x_sb = ptile("x_sb", [P, 2 * NPIX], F32)  # channel-block major
            pad_sb = ptile("pad_sb", [P, 2 * HB * HP * WP], FP8)
            w1_sb = ptile("w1_sb", [P, 9 * 2 * C], FP8)
            w2_sb = ptile("w2_sb", [P, 2 * C], FP8)
            sg2_sb = ptile("sg2_sb", [P, 2 * NPIX], FP8)
            u_pre = [ptile(f"u_pre{i}", [P, NPIX], F32) for i in range(2)]
            p1 = [ptile(f"p1_{i}", [P, NPIX], F32) for i in range(2)]
            v_pre = [ptile(f"v_pre{i}", [P, NPIX], F32) for i in range(2)]
            p2 = [ptile(f"p2_{i}", [P, NPIX], F32) for i in range(2)]

            def cc(blk, j):  # per-channel scalar AP [128,1]
                return c_sb[:, blk * NCN + j:blk * NCN + j + 1]

            def xsl(blk, sl):  # x slice [128, n] for channel block blk
                return x_sb[:, blk * NPIX + sl.start:blk * NPIX + sl.stop]

            # ---- loads, ordered so image 0 can start ASAP (consts ride the
            # scalar engine's HWDGE stream, off the critical sync queue) ----
            nc.scalar.dma_start(out=c_sb, in_=c_t[:])
            xv4 = x_sb.rearrange("p (j b q) -> p j b q", j=2, b=HB)
            xrows0 = x_sb.rearrange("p (j b h w) -> p j b h w",
                                    j=2, b=HB, h=H, w=W)
            # image 0 in two row-bands so Sign (and the first matmuls) can
            # start before the whole image lands
            x0_in = x_t[0].rearrange("(j p) h w -> p j h w", p=P)
            if X_SPLIT:
                nc.sync.dma_start(out=xrows0[:, :, 0, 0:15],
                                  in_=x0_in[:, :, 0:15])
                nc.sync.dma_start(out=xrows0[:, :, 0, 15:H],
                                  in_=x0_in[:, :, 15:H])
            else:
                # per channel block, so block 0's Sign starts ~1us before the
                # second half of image 0 lands
                for j in range(2):
                    nc.sync.dma_start(
                        out=xv4[:, j, 0],
                        in_=x_t[0, j * P:(j + 1) * P].rearrange(
                            "c h w -> c (h w)"))
            # first three kernel positions land first: the opening matmuls
            # only need w1v[:, 0..2] while the rest of the weights stream in
            nc.sync.dma_start(out=w1_sb[:, 0:3 * 2 * C], in_=w1_t[:, 0:3 * 2 * C])
            nc.sync.dma_start(out=w1_sb[:, 3 * 2 * C:], in_=w1_t[:, 3 * 2 * C:])
            for j in range(2):
                nc.sync.dma_start(
                    out=xv4[:, j, 1:HB],
                    in_=x_t[1:HB, j * P:(j + 1) * P].rearrange(
                        "b c h w -> c b (h w)"))
            nc.sync.dma_start(out=w2_sb, in_=w2_t[:])

            # warm the ACT function-table set (Copy shares the set with
            # Sign/Prelu) so the ~2.7us table load overlaps the input DMAs
            dummy = ptile("dummy", [P, 8], F32)
            nc.vector.memset(dummy[:, 0:4], 0.0)
            nc.scalar.activation(dummy[:, 4:8], dummy[:, 0:4], AF.Copy)

            pad5 = pad_sb.rearrange("p (j b h w) -> p j b h w",
                                    j=2, b=HB, h=HP, w=WP)
            pad_j = pad_sb.rearrange("p (j q) -> p j q", j=2)
            w1v = w1_sb.rearrange("p (k j o) -> p k j o", k=9, j=2)
            w2v = w2_sb.rearrange("p (j o) -> p j o", j=2)
            sg2v = sg2_sb.rearrange("p (j q) -> p j q", j=2)

            # zero only the pad borders, per image (interiors are overwritten
            # by Sign): top+bottom rows, then left+right columns
            for b in range(HB):
                for i in range(2):
                    nc.vector.memset(pad5[:, i, b, 0:HP:HP - 1, :], 0.0)
                    nc.vector.memset(pad5[:, i, b, 1:HP - 1, 0:WP:WP - 1], 0.0)

            # sgn1 = Sign(x + b11) written into padded interior, fp8.
            # Image 0 in two row-bands matching its split DMA.
            if X_SPLIT:
                for i in range(2):
                    nc.scalar.activation(
                        pad5[:, i, 0, 1:16, 1:1 + W], xrows0[:, i, 0, 0:15],
                        AF.Sign, bias=cc(i, J_B11))
                for i in range(2):
                    nc.scalar.activation(
                        pad5[:, i, 0, 16:1 + H, 1:1 + W], xrows0[:, i, 0, 15:H],
                        AF.Sign, bias=cc(i, J_B11))
            if not X_SPLIT and DVE_SIGN1:
                # image 0, block 0 on ACT; block 1 on DVE as
                # 2*[x >= -b11] - 1 so both blocks sign concurrently
                nc.scalar.activation(
                    pad5[:, 0, 0, 1:1 + H, 1:1 + W],
                    xsl(0, slice(0, IMG)).rearrange("p (h w) -> p h w", h=H),
                    AF.Sign, bias=cc(0, J_B11))
                sgntmp = ptile("sgntmp", [P, IMG], F32)
                nc.vector.tensor_scalar(sgntmp, xsl(1, slice(0, IMG)),
                                        cc(1, J_NB11), None, op0=OP.is_ge)
                nc.vector.tensor_scalar(
                    pad5[:, 1, 0, 1:1 + H, 1:1 + W],
                    sgntmp.rearrange("p (h w) -> p h w", h=H),
                    2.0, -1.0, op0=OP.mult, op1=OP.add)
            b0_range = range(1, HB) if (X_SPLIT or DVE_SIGN1) else range(HB)
            for b in b0_range:
                for i in range(2):
                    nc.scalar.activation(
                        pad5[:, i, b, 1:1 + H, 1:1 + W],
                        xsl(i, slice(b * IMG, (b + 1) * IMG)).rearrange(
                            "p (h w) -> p h w", h=H),
                        AF.Sign, bias=cc(i, J_B11))

            # chunk groups sharing stationary weights: singleton first/last
            # groups shorten the kernel head (image 0 prep only) and tail
            GROUPS = GROUPS_CFG

            def grp_chunks(grp):
                return [(b, half) for b in GROUPS[grp] for half in range(2)]

            def chunk_sl(b, half):
                return slice(b * IMG + half * CH, b * IMG + half * CH + CH)

            def r14x28(ap_flat_392):
                return ap_flat_392.rearrange("p (r c) -> p r c", c=W)

            def conv1_group(grp, ob, seq=False):
                chunks = grp_chunks(grp)
                # ps_0/ps_1 double-buffer across groups; ps_2/ps_3 are only
                # used by the 4-chunk middle groups (single-buffered) so two
                # banks stay free for conv2's dedicated tiles
                pss = [psp.tile([P, 420], F32, name=f"ps_{ci}",
                                tag=f"ps_{ci}", bufs=(2 if ci < 2 else 1))
                       for ci in range(len(chunks))]
                # seq=True: finish chunks one at a time (extra LDWEIGHTS, but
                # the per-chunk epilogue chain starts a weight-pass earlier)
                chunk_sets = ([[c] for c in chunks] if seq else [chunks])
                for cset in chunk_sets:
                    for k in range(9):
                        ky, kx = divmod(k, 3)
                        w = w1v[:, k, :, ob * P:(ob + 1) * P]
                        for b, half in cset:
                            ci = chunks.index((b, half))
                            off = b * HP * WP + (half * (H // 2) + ky) * WP + kx
                            nc.tensor.matmul(
                                pss[ci][:, 0:SEAM], w,
                                pad_j[:, :, off:off + SEAM],
                                start=(k == 0), stop=(k == 8), perf_mode=DR)
                for ci, (b, half) in enumerate(chunks):
                    # u_pre = s1 * psum + x  (valid 392 of the 418 columns)
                    psv = pss[ci].rearrange("p (r c) -> p r c", c=WP)[:, 0:14, 0:W]
                    sl = chunk_sl(b, half)
                    nc.vector.scalar_tensor_tensor(
                        r14x28(u_pre[ob][:, sl]), psv, cc(ob, J_S1),
                        r14x28(xsl(ob, sl)), op0=OP.mult, op1=OP.add)

            def branch1_ew(grp, fine):
                # branch-2 activation as a {0,1} indicator on GpSimd, straight
                # from u_pre (prelu1 is off this path): conv2(2u-1) is
                # recovered via s2'=2*s2 and the host-folded rowsum in t2b'.
                # Always chunk-grained: it gates the next conv2 group.
                for b, half in grp_chunks(grp):
                    sl = chunk_sl(b, half)
                    for ob in range(2):
                        nc.gpsimd.tensor_scalar(
                            sg2v[:, ob, sl], u_pre[ob][:, sl],
                            cc(ob, J_SG2B), 0.0, op0=OP.add, op1=OP.is_ge)
                # p1 = prelu(u_pre + (bn1_const + b12), a1): residual only
                if fine:
                    pieces = [chunk_sl(b, half) for b, half in grp_chunks(grp)]
                else:
                    pieces = [slice(b * IMG, (b + 1) * IMG) for b in GROUPS[grp]]
                for sl in pieces:
                    for ob in range(2):
                        nc.scalar.activation(p1[ob][:, sl], u_pre[ob][:, sl],
                                             AF.Prelu, bias=cc(ob, J_T1B),
                                             alpha=cc(ob, J_A1))

            def conv2_group(grp):
                chunks = grp_chunks(grp)
                for ob in range(2):
                    w = w2v[:, :, ob * P:(ob + 1) * P]
                    pss = [psp.tile([P, 420], F32, name=f"cps_{ci % 2}",
                                    tag=f"cps_{ci % 2}", bufs=1)
                           for ci in range(len(chunks))]
                    for ci, (b, half) in enumerate(chunks):
                        nc.tensor.matmul(
                            pss[ci][:, 0:CH], w, sg2v[:, :, chunk_sl(b, half)],
                            start=True, stop=True, perf_mode=DR)
                    for ci, (b, half) in enumerate(chunks):
                        # v_pre = s2 * psum + p1
                        sl = chunk_sl(b, half)
                        nc.vector.scalar_tensor_tensor(
                            v_pre[ob][:, sl], pss[ci][:, 0:CH], cc(ob, J_S2),
                            p1[ob][:, sl], op0=OP.mult, op1=OP.add)

            def branch2_ew(grp, fine):
                pieces = (grp_chunks(grp) if fine
                          else [(b, None) for b in GROUPS[grp]])
                for b, half in pieces:
                    sl = chunk_sl(b, half) if half is not None else \
                        slice(b * IMG, (b + 1) * IMG)
                    for ob in range(2):
                        # p2 = prelu(v_pre + (bn2_const + b22 + b13), a2)
                        nc.scalar.activation(
                            p2[ob][:, sl], v_pre[ob][:, sl],
                            AF.Prelu, bias=cc(ob, J_T2B),
                            alpha=cc(ob, J_A2))
                        # out = p2 + b23, staged into x_sb (x is dead by now)
                        eng = (nc.vector if LAST_ADD_DVE
                               and grp == len(GROUPS) - 1 else nc.gpsimd)
                        eng.tensor_scalar_add(xsl(ob, sl),
                                              p2[ob][:, sl], cc(ob, J_B23))
                    hw0 = 0 if half is None else half * (H // 2)
                    nrows = H if half is None else H // 2
                    if half is None:
                        # one DMA per image covering both channel blocks
                        xq = x_sb.rearrange("p (j q) -> p j q", j=2)
                        nc.sync.dma_start(
                            out=out_t[b, :, hw0:hw0 + nrows].rearrange(
                                "(j p) h w -> p j (h w)", p=P),
                            in_=xq[:, :, sl.start:sl.stop])
                    else:
                        # tail pieces ship per channel block, as soon as ready
                        for ob in range(2):
                            nc.sync.dma_start(
                                out=out_t[b, ob * P:(ob + 1) * P,
                                          hw0:hw0 + nrows].rearrange(
                                    "c h w -> c (h w)"),
                                in_=xsl(ob, sl))

            # schedule: each conv2 group is sandwiched inside the next conv1
            # group so the PE never waits on the elementwise chain; the final
            # group (image 3 only) runs fine-grained (392-px pieces) to
            # shorten the kernel tail.
            ops = {
                "c1": conv1_group,
                "c2": conv2_group,
                "e1": branch1_ew,
                "e2": branch2_ew,
            }
            for step in SCHEDULE:
                ops[step[0]](*step[1:])

    _dedup_ldweights(nc)
    nc.compile()
    return nc


def _dedup_ldweights(nc):
    """Drop InstLdweights that reload the exact weights already resident in
    the PE array (weight-reuse groups emit one matmul per pixel chunk, and the
    tile lowering pairs every matmul with its own Ldweights). Only wait-free,
    update-free duplicates are removed, so semaphore semantics are untouched."""
    pe = nc.tensor.engine
    removed = 0
    for blk in nc.m.functions[0].blocks:
        last_sig = None
        keep = []
        for ins in blk.instructions:
            if ins.engine == pe:
                t = type(ins).__name__
                if t == "InstLdweights":
                    sig = repr(ins.ins[0])
                    if (sig == last_sig and not ins.has_wait()
                            and not ins.has_update()):
                        removed += 1
                        continue
                    last_sig = sig
                elif t not in ("InstMatmult", "InstEventSemaphore"):
                    last_sig = None
            keep.append(ins)
        if removed:
            while len(blk.instructions):
                blk.instructions.pop()
            for ins in keep:
                blk.instructions.append(ins)
    return removed


def _prep_params(w1, w2, b11, b12, b13, b21, b22, b23,
                 g1, be1, m1, v1, g2, be2, m2, v2, a1, a2):
    f = np.float32
    eps = f(1e-5)
    scale1 = np.mean(np.abs(w1), axis=(1, 2, 3), dtype=np.float32)
    scale2 = np.mean(np.abs(w2), axis=(1, 2, 3), dtype=np.float32)
    inv1 = (g1 / np.sqrt(v1 + eps)).astype(f)
    inv2 = (g2 / np.sqrt(v2 + eps)).astype(f)
    s1 = (scale1 * inv1).astype(f)
    s2 = (scale2 * inv2).astype(f)
    t1b = ((be1 - m1 * inv1) + b12).astype(f)
    c2 = (b13 + b21).astype(f)
    t2b = (((be2 - m2 * inv2) + b22) + b13).astype(f)
    # branch-2 activations are sent as indicators u in {0,1}:
    # conv2(2u-1)[o] = 2*conv2(u)[o] - rowsum(sign(w2))[o]
    rs2 = np.sign(w2[:, :, 0, 0]).astype(f).sum(axis=1, dtype=np.float32)
    s2_eff = (2.0 * s2).astype(f)
    t2b_eff = (t2b - s2 * rs2).astype(f)
    # sign2 threshold straight from u_pre: sign(prelu(t)+c2) = [t >= t*],
    # t* = -c2 (c2<=0) else -c2/a1  (prelu is monotone for a1>0)
    a1_safe = np.where(a1 != 0, a1, f(1.0)).astype(f)
    t_star = np.where(c2 <= 0, -c2, -(c2 / a1_safe)).astype(f)
    sg2b = (t1b - t_star).astype(f)

    consts = np.stack([b11, s1, t1b, a1, sg2b, s2_eff, t2b_eff, a2, b23,
                       -b11], axis=1).astype(f).reshape(2, P, NCN)
    consts = np.ascontiguousarray(consts.transpose(1, 0, 2).reshape(P, 2 * NCN))

    fp8 = ml_dtypes.float8_e4m3
    # w1t[p, k, j, o] = sign(w1)[o, j*128+p, ky, kx]
    w1t = np.sign(w1).astype(np.float32).transpose(1, 2, 3, 0)  # [I,ky,kx,O]
    w1t = w1t.reshape(2, P, 9, C).transpose(1, 2, 0, 3)          # [p,k,j,o]
    w1t = np.ascontiguousarray(w1t.reshape(P, 9 * 2 * C)).astype(fp8)
    # w2t[p, j, o] = sign(w2)[o, j*128+p]
    w2t = np.sign(w2[:, :, 0, 0]).astype(np.float32).T.reshape(2, P, C)
    w2t = np.ascontiguousarray(w2t.transpose(1, 0, 2).reshape(P, 2 * C)).astype(fp8)
    return consts, w1t, w2t


def kernel(x, loss, b11, b12, b13, b21, b22, b23, w1, w2,
           g1, be1, m1, v1, g2, be2, m2, v2, a1, a2):
    x = np.ascontiguousarray(np.asarray(x, np.float32))
    args = [np.asarray(a, np.float32) for a in
            (w1, w2, b11, b12, b13, b21, b22, b23,
             g1, be1, m1, v1, g2, be2, m2, v2, a1, a2)]
    consts, w1t, w2t = _prep_params(*args)

    if "nc" not in _CACHE:
        _CACHE["nc"] = _build_nc()
    nc = _CACHE["nc"]

    in_maps = []
    for core in range(NCORES):
        in_maps.append({
            "x": np.ascontiguousarray(x[core * HB:(core + 1) * HB]),
            "w1t": w1t,
            "w2t": w2t,
            "consts": consts,
        })
    res = run_bass_kernel_spmd(nc, in_maps, core_ids=list(range(NCORES)))
    out = np.concatenate([r["out"] for r in res.results], axis=0)
    return (out, np.asarray(loss, np.float32))
